# revision 8
# baseline (speedup 1.0000x reference)
# Trainium2 Bass kernel for nn_MeshUnpool (gnn_message_passing).
#
# Reference semantics (per mesh b):
#   idx = cumsum(dst_mask)-1 at true slots; padded[v,:] = mask[v] ? features[:,idx[v]] : 0
#   out = (unroll_mat[b].T @ padded).T / occ  ==  (features[b] @ unroll_mat[b][mask_rows]) / occ
# i.e. the gather+scatter collapses to selecting the E=3072 masked rows of
# unroll_mat, leaving a dense [NF,E] @ [E,U] matmul per mesh, divided
# column-wise by occurrences.  Pure data parallel: one mesh per core.
#
# On-device compute per core:
#   out[128, 4096] = sum_k (A_hi[k] + A_lo[k]).T @ W[k]  * inv_occ
# where A_hi/A_lo is a bf16 hi/lo split of features^T (f32-grade accuracy,
# since bf16*bf16 products are exact in the f32 PSUM accumulator) and W is the
# masked-row-gathered unroll matrix cast to bf16 (entries are exactly 0/1, so
# the cast is lossless and halves the dominant HBM traffic).  All-zero W rows
# (~6%) are dropped on host, shrinking the contraction further.

import numpy as np
import ml_dtypes

B, NF, E, U = 8, 128, 3072, 4096
NCORES = 8
NT = U // 512          # 8 output column tiles of 512 (one PSUM bank each)

_compiled = {}


def _build_bass(kc):
    """Build + compile the per-core program for a contraction of kc*128 rows."""
    import concourse.bass as bass
    import concourse.bacc as bacc
    import concourse.mybir as mybir
    import concourse.tile as tile

    e = kc * 128
    nc = bacc.Bacc("TRN2", target_bir_lowering=False, debug=False)
    bf16 = mybir.dt.bfloat16
    f32 = mybir.dt.float32

    a_hi = nc.dram_tensor("a_hi", [128, e], bf16, kind="ExternalInput").ap()
    a_lo = nc.dram_tensor("a_lo", [128, e], bf16, kind="ExternalInput").ap()
    w = nc.dram_tensor("w", [e, U], bf16, kind="ExternalInput").ap()
    occ = nc.dram_tensor("occ", [128, U], f32, kind="ExternalInput").ap()
    out = nc.dram_tensor("out", [128, U], f32, kind="ExternalOutput").ap()

    with tile.TileContext(nc) as tc:
        with (
            tc.tile_pool(name="const", bufs=1) as cpool,
            tc.tile_pool(name="wpool", bufs=8) as wpool,
            tc.tile_pool(name="psum", bufs=1, space=bass.MemorySpace.PSUM) as ppool,
            tc.tile_pool(name="opool", bufs=3) as opool,
        ):
            # stationary operands on the scalar HWDGE ring so the sync ring
            # streams W exclusively.  First chunk of a_hi goes separately so
            # the first matmul doesn't wait for the whole 0.75MB.
            a_hi_s = cpool.tile([128, e], bf16, tag="ahi")
            a_lo_s = cpool.tile([128, e], bf16, tag="alo")
            occ_s = cpool.tile([128, U], f32, tag="occ")
            nc.scalar.dma_start(a_hi_s[:, 0:128], a_hi[:, 0:128])
            nc.scalar.dma_start(a_hi_s[:, 128:e], a_hi[:, 128:e])
            nc.scalar.dma_start(a_lo_s[:], a_lo)

            # all 8 PSUM banks accumulate in parallel; k-contiguous keeps PE warm
            psum = ppool.tile([128, U], f32, tag="ps")
            # host ships A^T chunk-interleaved: a_hi[p, k*128+m] = AT[k*128+p, m]
            # so chunk k's lhsT [K=128, M=128] is a_hi_s[:, k*128:(k+1)*128]
            w_last = None
            for k in range(kc):
                w_t = wpool.tile([128, U], bf16, tag="w")
                if k == 0:
                    # per-bank pieces: the first matmul only waits for 128KB
                    for n in range(NT):
                        nc.sync.dma_start(
                            w_t[:, n * 512 : (n + 1) * 512],
                            w[0:128, n * 512 : (n + 1) * 512],
                        )
                else:
                    nc.sync.dma_start(w_t[:], w[k * 128 : (k + 1) * 128, :])
                if k == kc // 2:
                    # occ is only needed for the epilogue; mid-stream the DMA
                    # slack behind the PE-bound phase absorbs it for free
                    nc.scalar.dma_start(occ_s[:], occ)
                last = k == kc - 1
                # hi half always; lo half of the last chunk moves to the
                # epilogue below so mul+store can interleave per bank
                for half in range(1 if last else 2):
                    a_s = a_hi_s if half == 0 else a_lo_s
                    for n in range(NT):
                        nc.tensor.matmul(
                            psum[:, n * 512 : (n + 1) * 512],
                            a_s[:, k * 128 : (k + 1) * 128],
                            w_t[:, n * 512 : (n + 1) * 512],
                            start=(k == 0 and half == 0),
                            stop=False,
                        )
                w_last = w_t
            # final lo pass: finish each bank then immediately scale+store it,
            # so the epilogue overlaps the remaining banks' matmuls
            k = kc - 1
            for n in range(NT):
                nc.tensor.matmul(
                    psum[:, n * 512 : (n + 1) * 512],
                    a_lo_s[:, k * 128 : (k + 1) * 128],
                    w_last[:, n * 512 : (n + 1) * 512],
                    start=False,
                    stop=True,
                )
                o_t = opool.tile([128, 512], f32, tag="o")
                nc.vector.tensor_mul(
                    o_t[:], psum[:, n * 512 : (n + 1) * 512],
                    occ_s[:, n * 512 : (n + 1) * 512],
                )
                nc.sync.dma_start(out[:, n * 512 : (n + 1) * 512], o_t[:])

    nc.compile()
    _dedup_ldweights(nc)
    return nc


def _dedup_ldweights(nc):
    """Remove InstLdweights that reload the PE array with the exact weights it
    already holds (consecutive matmuls sharing one stationary operand).  The
    tile legalizer emits one LDWEIGHTS per matmul and neither it nor walrus
    dedups, so 8-matmul groups sharing a lhsT pay 7 redundant ~100ns array
    loads each — pure serial PE time.  Safe here because the stationary tiles
    (bufs=1, written once) are never rewritten mid-kernel.  Any waits/updates
    on a removed LDW are transferred to the next PE instruction."""
    import concourse.mybir as mybir

    for blk in nc.m.functions[0].blocks:
        insts = blk.instructions
        loaded = None
        pending = []  # sync infos of removed LDWs, to merge into next PE inst
        idx = 0
        while idx < len(insts):
            inst = insts[idx]
            if isinstance(inst, mybir.InstLdweights):
                key = (
                    str(inst.ins[0]),
                    str(inst.tile_position),
                    str(inst.perf_mode),
                    str(inst.is_transpose),
                )
                if loaded == key:
                    si = inst.sync_info
                    if si is not None and (si.on_wait or si.on_update):
                        pending.append(si)
                    del insts[idx]
                    continue
                loaded = key
            elif isinstance(inst, mybir.InstMatmult) and pending:
                si = inst.sync_info
                if si is None:
                    si = mybir.SyncInfo(on_wait=[], on_update=[])
                for p in pending:
                    si.on_wait = list(si.on_wait) + list(p.on_wait)
                    si.on_update = list(si.on_update) + list(p.on_update)
                inst.sync_info = si
                pending = []
            idx += 1
        assert not pending, "dangling sync from removed LDWEIGHTS"


def _get_compiled(kc):
    if kc not in _compiled:
        _compiled[kc] = _build_bass(kc)
    return _compiled[kc]


def _prep_cores(features, unroll_mat, occurrences, dst_masks):
    """Host-side prep: mask-gather W rows, drop all-zero rows, hi/lo split of
    features^T, 1/occ broadcast.  Returns (kc, in_maps)."""
    bf16 = ml_dtypes.bfloat16
    per_core = []
    for b in range(B):
        wg = unroll_mat[b][dst_masks[b]]          # [E, U] f32, entries 0/1
        keep = wg.any(axis=1)                      # drop rows with no targets
        wk = wg[keep]
        fk = features[b][:, keep]                  # matching feature columns
        per_core.append((wk, fk))
    kmax = max(w_.shape[0] for w_, _ in per_core)
    kc = (kmax + 127) // 128
    e = kc * 128

    in_maps = []
    for b in range(B):
        wk, fk = per_core[b]
        r = wk.shape[0]
        wpad = np.zeros((e, U), dtype=bf16)
        wpad[:r] = wk.astype(bf16)                 # 0/1 -> exact
        at = np.zeros((e, 128), dtype=np.float32)  # A^T, zero-padded rows
        at[:r] = fk.T
        hi = at.astype(bf16)
        lo = (at - hi.astype(np.float32)).astype(bf16)

        def interleave(x):  # [e,128] -> [128,e]; col k*128+m holds x[k*128+p, m]
            return np.ascontiguousarray(
                x.reshape(kc, 128, 128).transpose(1, 0, 2).reshape(128, e)
            )

        inv_occ = (1.0 / occurrences[b].reshape(U).astype(np.float32)).astype(
            np.float32
        )
        in_maps.append(
            {
                "a_hi": interleave(hi),
                "a_lo": interleave(lo),
                "w": wpad,
                "occ": np.ascontiguousarray(np.broadcast_to(inv_occ, (128, U))),
            }
        )
    return kc, in_maps


def kernel(features, unroll_mat, occurrences, dst_masks):
    import concourse.bass_utils as bass_utils

    features = np.asarray(features, dtype=np.float32)
    unroll_mat = np.asarray(unroll_mat, dtype=np.float32)
    occurrences = np.asarray(occurrences, dtype=np.float32)
    dst_masks = np.asarray(dst_masks).astype(bool)

    kc, in_maps = _prep_cores(features, unroll_mat, occurrences, dst_masks)
    nc = _get_compiled(kc)
    res = bass_utils.run_bass_kernel_spmd(nc, in_maps, core_ids=list(range(NCORES)))
    return np.stack([res.results[b]["out"] for b in range(B)], axis=0)
